# revision 7
# baseline (speedup 1.0000x reference)
"""MoE (GPT MLP, top-2, GShard capacity) kernel for 8 Trainium2 NeuronCores.

v5: w1 SBUF-resident, w2 streamed in 2MB chunks, 1024-token segments,
compound per-bank matmul emission with redundant-LDWEIGHTS removal, and
minimal DMA-in descriptor count.

Measured hardware facts driving this design (probes on this device):
  - bf16 matmuls stream ~2 moving rows/cycle (512-row MM ~107-130 ns);
    2048 MMs of 512 rows are the PE floor (~270 us/pass).
  - An extra LDWEIGHTS instruction costs ~90 ns; tile_legalize emits one
    per matmul, so back-to-back same-stationary matmuls are deduped
    post-compile (_dedup_ldweights).
  - A DMA-in descriptor active during PE work costs ~10 us of PE
    throughput regardless of its size (1-4MB); DMA-out is ~4x cheaper.
    So: w1 resident (zero steady-state traffic), w2 in 4 big chunks per
    segment, x as one 4MB descriptor, 10 in-descriptors/pass total.

Strategy (expert-parallel, matching the sharding hint):
  - Host: fp32 gate (softmax + top-2 + GShard capacity positions),
    dispatch gather.  Routing is O(N*E) int/scalar work - negligible next
    to the FFN - and the capacity scan is inherently sequential.
  - Device: 8 cores, core e owns expert e (cap=2048 token slots).
    Per 1024-token segment: phase A accumulates 8 compound matmul chains
    per H-tile into 2 PSUM banks, gelu+bias drains to bf16 h; phase B
    contracts h against streamed w2 chunks.
  - Host: combine (gather + gate-weighted sum) + b2.

Self-contained: hardcodes B=4, S=2048, D=1024, H=4096, E=8, K=2, cap=2048.
"""

import sys

sys.path.insert(0, "/opt/trn_rl_repo")

import numpy as np
import ml_dtypes

B, S, D, H, E = 4, 2048, 1024, 4096, 8
K = 2
N_TOK = B * S            # 8192
CAP = (K * N_TOK) // E   # 2048 (capacity factor 1.0)
EPS = 1e-9
P = 128                  # SBUF partitions

BF16 = ml_dtypes.bfloat16

_NC_CACHE = {}


# --------------------------------------------------------------------------
# Host routing (replicates reference.py's gate exactly, in numpy fp32)
# --------------------------------------------------------------------------

def _route(xt, wg):
    """xt: [N, D] fp32, wg: [D, E] fp32 ->
    gidx [N,K] int, gvals [N,K] fp32 (keep-masked), pos [N,K] int, keep [N,K]"""
    logits = xt @ wg                                   # [N, E] fp32
    m = logits.max(axis=-1, keepdims=True)
    ex = np.exp(logits - m)
    scores = ex / ex.sum(axis=-1, keepdims=True)
    order = np.argsort(-scores, axis=1, kind="stable")  # jax top_k tie rule
    gidx = order[:, :K]                                 # [N, K]
    gvals = np.take_along_axis(scores, gidx, axis=1)
    gvals = gvals / np.clip(gvals.sum(-1, keepdims=True), EPS, None)

    n = xt.shape[0]
    offset = np.zeros(E, np.int64)
    pos = np.zeros((n, K), np.int64)
    keep = np.zeros((n, K), bool)
    rows = np.arange(n)
    for kk in range(K):
        ek = gidx[:, kk]
        oh = np.zeros((n, E), np.int64)
        oh[rows, ek] = 1
        loc = np.cumsum(oh, axis=0) - 1 + offset[None, :]
        offset = offset + oh.sum(axis=0)
        p = loc[rows, ek]
        kmask = p < CAP
        pos[:, kk] = np.where(kmask, p, 0)
        keep[:, kk] = kmask
    gvals = (gvals * keep).astype(np.float32)
    return gidx, gvals, pos, keep


# --------------------------------------------------------------------------
# LDWEIGHTS dedup (post-compile IR pass)
# --------------------------------------------------------------------------

def _ldw_key(inst):
    """Identity key for an InstLdweights: the physical weights AP plus the
    load-mode flags.  Two consecutive LDWs with equal keys load identical
    PE-array contents (same SBUF address/shape; no DMA can rewrite that
    address between them without an intervening differently-keyed LDW in
    this kernel's emission order, because weight-pool buffers are only
    recycled after other chains' LDWs)."""
    return (str(inst.ins[0]), inst.perf_mode, inst.is_transpose,
            str(inst.tile_position), str(inst.tile_size))


def _dedup_ldweights(nc):
    """Remove redundant PE weight loads after compilation.

    tile_legalize splits every bf16 matmul into InstLdweights +
    InstMatmult(ldweights=False).  Matmuls emitted back-to-back against
    the same stationary block (the per-bank copies of one compound
    matmul) therefore reload the PE array each time; all but the first
    load are no-ops.  Drop an InstLdweights when (a) the previous
    PE-queue instruction sequence since the last kept LDW contains only
    non-self-loading matmuls / event semaphores and that LDW has an
    identical key, and (b) it carries no semaphore waits or updates (the
    chain-leading LDW, which inherits the weight-DMA wait, always
    differs in key from its predecessor and is kept).
    """
    from concourse import mybir

    removed = 0
    for blk in nc.main_func.blocks:
        insts = list(blk.instructions)
        keep = []
        last_key = None
        for inst in insts:
            if isinstance(inst, mybir.InstLdweights):
                si = inst.sync_info
                clean = si is None or (len(si.on_wait) == 0
                                       and len(si.on_update) == 0)
                key = _ldw_key(inst)
                if clean and key == last_key:
                    removed += 1
                    continue
                last_key = key
            elif isinstance(inst, mybir.InstMatmult):
                if inst.ldweights is not False:
                    last_key = None        # self-loading: clobbers array
            elif isinstance(inst, mybir.InstEventSemaphore):
                pass                       # pure sync: array untouched
            elif getattr(inst, "engine", None) == mybir.EngineType.PE:
                last_key = None            # unknown PE op: be conservative
            keep.append(inst)
        if removed and len(keep) != len(insts):
            while len(blk.instructions):
                blk.instructions.pop()
            for inst in keep:
                blk.instructions.append(inst)
    return removed


# --------------------------------------------------------------------------
# Device kernel builder (one expert FFN per core, SPMD)
# --------------------------------------------------------------------------

def _build_nc(d, h, ntok, debug=False, act="Gelu", reps=1, loop_trip=None):
    """Expert FFN: y[ntok, d] = gelu(x[ntok, d] @ w1[d, h] + b1[h]) @ w2[h, d].

    Device inputs:
      xb  : [P, d/P, 2, ntok/2] bf16   x[s*seg+t, j*P+p] at [p, j, s, t]
      w1b : [P, d/P, h/P, P] bf16      w1[j*P+p, m*P+c] at [p, j, m, c]
      w2b : [4, P, h/P, 2*P] bf16      w2[m*P+p, db*P+cc] at
                                       [db//2, p, m, (db%2)*P+cc]
      b1t : [P, h/P] fp32              b1 transposed
    Output:
      y   : [d/P, P, 2, ntok/2] bf16   y[s*seg+t, db*P+p] at [db, p, s, t]
    """
    import contextlib

    from concourse import bacc, mybir, tile

    dt_n = d // P            # 8
    mt_n = h // P            # 32
    db_n = d // P            # 8
    seg_n = 2                # token segments
    seg = ntok // seg_n      # 1024
    bk_n = seg // 512        # 2 PSUM banks per compound matmul
    c2 = 2                   # dbs per w2 chunk (2MB descriptors)

    f32 = mybir.dt.float32
    bf = mybir.dt.bfloat16
    actf = getattr(mybir.ActivationFunctionType, act)

    nc = bacc.Bacc("TRN2", target_bir_lowering=False, debug=debug,
                   enable_asserts=False, num_devices=1)

    xb_d = nc.dram_tensor("xb", [P, dt_n, seg_n, seg], bf,
                          kind="ExternalInput")
    w1_d = nc.dram_tensor("w1b", [P, dt_n, mt_n, P], bf, kind="ExternalInput")
    w2_d = nc.dram_tensor("w2b", [db_n // c2, P, mt_n, c2 * P], bf,
                          kind="ExternalInput")
    b1_d = nc.dram_tensor("b1t", [P, mt_n], f32, kind="ExternalInput")
    y_d = nc.dram_tensor("y", [db_n, P, seg_n, seg], bf,
                         kind="ExternalOutput")

    with tile.TileContext(nc) as tc:
        with (
            tc.tile_pool(name="cpool", bufs=1) as cpool,
            tc.tile_pool(name="w2pool", bufs=2) as w2pool,
            tc.tile_pool(name="ypool", bufs=2) as ypool,
            tc.tile_pool(name="ps", bufs=2, space="PSUM") as pspool,
        ):
            b1_t = cpool.tile([P, mt_n], f32, name="b1_t")
            w1_t = cpool.tile([P, dt_n, mt_n, P], bf, name="w1_t")
            x_t = cpool.tile([P, dt_n, seg_n, bk_n, 512], bf, name="x_t")
            h_t = cpool.tile([P, mt_n, bk_n, 512], bf, name="h_t")
            nc.sync.dma_start(b1_t[:], b1_d[:])
            nc.sync.dma_start(w1_t[:], w1_d[:])   # resident; loaded once

            # loop_trip: hardware For_i loop around the rep body (used for
            # low-variance timing: small program, long execution)
            loop_cm = (tc.For_i(0, loop_trip) if loop_trip
                       else contextlib.nullcontext())
            with loop_cm:
              for rep in range(reps):
                nc.sync.dma_start(x_t[:], xb_d[:])   # one 4MB descriptor
                for s in range(seg_n):
                    # ---- phase A: h = gelu(x_seg @ w1 + b1) ----
                    for m in range(mt_n):
                        ps = pspool.tile([P, bk_n, 512], f32, tag="ps",
                                         name=f"ps1_{rep}_{s}_{m}")
                        for j in range(dt_n):
                            for k in range(bk_n):
                                nc.tensor.matmul(
                                    ps[:, k],
                                    w1_t[:, j, m, :],
                                    x_t[:, j, s, k],
                                    start=(j == 0),
                                    stop=(j == dt_n - 1),
                                )
                        nc.scalar.activation(
                            h_t[:, m], ps[:], actf,
                            bias=b1_t[:, m:m + 1], scale=1.0,
                        )
                    # ---- phase B: y_seg = h @ w2 (w2 streamed in chunks) --
                    for c in range(db_n // c2):
                        w2t = w2pool.tile([P, mt_n, c2 * P], bf, tag="w2",
                                          name=f"w2_{rep}_{s}_{c}")
                        nc.sync.dma_start(w2t[:], w2_d[c])
                        for dc in range(c2):
                            db = c * c2 + dc
                            ps = pspool.tile([P, bk_n, 512], f32, tag="ps",
                                             name=f"ps2_{rep}_{s}_{db}")
                            for m in range(mt_n):
                                for k in range(bk_n):
                                    nc.tensor.matmul(
                                        ps[:, k],
                                        w2t[:, m, dc * P:(dc + 1) * P],
                                        h_t[:, m, k],
                                        start=(m == 0),
                                        stop=(m == mt_n - 1),
                                    )
                            yt = ypool.tile([P, seg], bf, tag="yt",
                                            name=f"yt_{rep}_{s}_{db}")
                            nc.vector.tensor_copy(yt[:], ps[:])
                            nc.sync.dma_start(y_d[db, :, s], yt[:])

    nc.compile()
    _dedup_ldweights(nc)
    return nc


def _get_nc(d, h, ntok, debug=False, reps=1, loop_trip=None):
    key = (d, h, ntok, debug, reps, loop_trip)
    if key not in _NC_CACHE:
        _NC_CACHE[key] = _build_nc(d, h, ntok, debug, reps=reps,
                                   loop_trip=loop_trip)
    return _NC_CACHE[key]


# --------------------------------------------------------------------------
# Host-side input layout per core
# --------------------------------------------------------------------------

def _core_inputs(disp_e, w1_e, w2_e, b1_e):
    """disp_e: [CAP, D], w1_e: [D, H], w2_e: [H, D], b1_e: [H]."""
    xb = np.ascontiguousarray(
        disp_e.T.astype(BF16).reshape(D // P, P, 2, CAP // 2)
        .transpose(1, 0, 2, 3))
    w1b = np.ascontiguousarray(
        w1_e.astype(BF16).reshape(D // P, P, H // P, P).transpose(1, 0, 2, 3))
    w2b = np.ascontiguousarray(
        w2_e.astype(BF16).reshape(H // P, P, D // (2 * P), 2 * P)
        .transpose(2, 1, 0, 3))
    b1t = np.ascontiguousarray(b1_e.reshape(H // P, P).T)
    return {"xb": xb, "w1b": w1b, "w2b": w2b, "b1t": b1t}


def _get_runner(nc, n_cores):
    """Cached PJRT executable for an SPMD bass program (axon path of
    run_bass_kernel_spmd, with the jitted callable kept warm across calls)."""
    key = id(nc)
    if key in _NC_CACHE:
        return _NC_CACHE[key]

    import jax
    from jax.sharding import Mesh, PartitionSpec
    from jax.experimental.shard_map import shard_map
    from concourse import mybir
    from concourse.bass2jax import (_bass_exec_p, install_neuronx_cc_hook,
                                    partition_id_tensor)

    install_neuronx_cc_hook()

    partition_name = (nc.partition_id_tensor.name
                      if nc.partition_id_tensor else None)
    in_names, out_names, out_avals = [], [], []
    for alloc in nc.m.functions[0].allocations:
        if not isinstance(alloc, mybir.MemoryLocationSet):
            continue
        name = alloc.memorylocations[0].name
        if alloc.kind == "ExternalInput":
            if name != partition_name:
                in_names.append(name)
        elif alloc.kind == "ExternalOutput":
            out_names.append(name)
            shape = tuple(alloc.tensor_shape)
            out_avals.append(jax.core.ShapedArray(shape, mybir.dt.np(alloc.dtype)))
    n_params = len(in_names)
    n_outs = len(out_avals)
    in_names = in_names + out_names
    if partition_name is not None:
        in_names.append(partition_name)
    donate = tuple(range(n_params, n_params + n_outs))

    def _body(*args):
        operands = list(args)
        if partition_name is not None:
            operands.append(partition_id_tensor())
        outs = _bass_exec_p.bind(
            *operands,
            out_avals=tuple(out_avals),
            in_names=tuple(in_names),
            out_names=tuple(out_names),
            lowering_input_output_aliases=(),
            sim_require_finite=True,
            sim_require_nnan=True,
            nc=nc,
        )
        return tuple(outs)

    devices = jax.devices()[:n_cores]
    mesh = Mesh(np.asarray(devices), ("core",))
    in_specs = (PartitionSpec("core"),) * (n_params + n_outs)
    out_specs = (PartitionSpec("core"),) * n_outs
    sharded = jax.jit(
        shard_map(_body, mesh=mesh, in_specs=in_specs, out_specs=out_specs,
                  check_rep=False),
        donate_argnums=donate, keep_unused=True,
    )

    def run(in_maps, reps=1, time_reps=False):
        import time as _time
        concat_in = [
            np.concatenate([np.asarray(m[in_names[i]]) for m in in_maps], axis=0)
            for i in range(n_params)
        ]
        concat_in = [jax.device_put(a) for a in concat_in]
        zero_sets = []
        for _ in range(reps):
            zero_sets.append([
                jax.device_put(np.zeros((n_cores * av.shape[0], *av.shape[1:]),
                                        av.dtype))
                for av in out_avals
            ])
        for zs in zero_sets:
            for z in zs:
                z.block_until_ready()
        for a in concat_in:
            a.block_until_ready()
        times = []
        out_arrs = None
        for r in range(reps):
            t0 = _time.perf_counter()
            out_arrs = sharded(*concat_in, *zero_sets[r])
            for o in out_arrs:
                o.block_until_ready()
            times.append(_time.perf_counter() - t0)
        results = [
            {name: np.asarray(out_arrs[i]).reshape(n_cores, *out_avals[i].shape)[c]
             for i, name in enumerate(out_names)}
            for c in range(n_cores)
        ]
        if time_reps:
            return results, times
        return results

    _NC_CACHE[key] = run
    return run


def _make_in_maps(x, wg, w1, b1, w2):
    xt = x.reshape(N_TOK, D)
    gidx, gvals, pos, keep = _route(xt, wg)
    disp = np.zeros((E, CAP, D), np.float32)
    for kk in range(K):
        tok = np.nonzero(keep[:, kk])[0]
        disp[gidx[tok, kk], pos[tok, kk]] = xt[tok]
    in_maps = [_core_inputs(disp[e], w1[e], w2[e], b1[e]) for e in range(E)]
    return in_maps, gidx, gvals, pos


def kernel(x, wg, w1, b1, w2, b2):
    x = np.asarray(x, np.float32)
    wg = np.asarray(wg, np.float32)
    w1 = np.asarray(w1, np.float32)
    b1 = np.asarray(b1, np.float32)
    w2 = np.asarray(w2, np.float32)
    b2 = np.asarray(b2, np.float32)

    in_maps, gidx, gvals, pos = _make_in_maps(x, wg, w1, b1, w2)

    nc = _get_nc(D, H, CAP)
    run = _get_runner(nc, E)
    results = run(in_maps)
    # device y is [db, p, s, t]; token-major per expert is [s*seg+t, db*P+p]
    y_all = np.stack([r["y"].astype(np.float32).transpose(2, 3, 0, 1)
                      .reshape(CAP, D) for r in results])  # [E,CAP,D]

    # combine: out = sum_k gvals * (y[e, pos] + b2[e])
    e_flat = gidx.reshape(-1)
    p_flat = pos.reshape(-1)
    yk = y_all[e_flat, p_flat] + b2[e_flat]
    w = gvals.reshape(-1).astype(np.float32)
    out = (yk * w[:, None]).reshape(N_TOK, K, D).sum(axis=1)
    return out.reshape(B, S, D).astype(np.float32)


# --------------------------------------------------------------------------
# Benchmarking helpers (test.py only)
# --------------------------------------------------------------------------

def bench_loop(x, wg, w1, b1, w2, b2, trips=(4, 504), calls=6, body_reps=2):
    """Per-pass device time via hardware-loop (For_i) trip-count slope.

    The program body is one full FFN pass (x/w2 DMA in, matmul1, gelu,
    matmul2, y DMA out); the loop repeats it trip times on-device.  The
    wall-time difference between trip counts divides out per-call host and
    tunnel overhead (~100 ms, +-10 ms) over hundreds of passes, giving a
    low-variance per-pass estimate.  All per-pass DMAs (x in, w2 stream,
    y out) are inside the loop body, exactly as in kernel(); w1 stays
    SBUF-resident across passes, as in kernel().

    Returns (per_pass_seconds, {trip: [wall_times]}).
    """
    x = np.asarray(x, np.float32)
    in_maps, _, _, _ = _make_in_maps(
        x, np.asarray(wg, np.float32), np.asarray(w1, np.float32),
        np.asarray(b1, np.float32), np.asarray(w2, np.float32))
    walls = {}
    for trip in trips:
        nc = _get_nc(D, H, CAP, reps=body_reps, loop_trip=trip)
        run = _get_runner(nc, E)
        _, t = run(in_maps, reps=calls, time_reps=True)
        walls[trip] = t
    lo = min(walls[trips[0]][1:])
    hi = min(walls[trips[1]][1:])
    per_pass = (hi - lo) / ((trips[1] - trips[0]) * body_reps)
    return per_pass, walls
